# revision 38
# baseline (speedup 1.0000x reference)
"""Trainium2 Bass kernel for nn_CaC_50637664420271.

Computes, for x:[16,256,64,64]:
  feat_k = wk @ x + bk  (1x1 conv), feat_q = wq @ x + bq
  krnl[n,c,3,3] = bmm(feat_k, feat_q^T)  -> BatchNorm (train stats) ->
  out = mean_d sigmoid(depthwise_conv(x, krnl, dilation=d)), d in {1,2,3}

Sharding: pure data-parallel over batch (2 samples / core, 8 cores), with a
tiny AllReduce of per-channel (sum, sumsq) of krnl for the BN batch stats.

Matmuls run as 3 bf16 products (x_hi*w_hi + x_hi*w_lo + x_lo*w_hi) of
host-prepared hi/lo splits: exact to ~2^-17, and 3 PE cycles/column instead
of fp32's 4. Depthwise conv: channels on partitions; each tap is a
per-channel scale of a shifted window of the zero-padded image. Taps are
split across the TensorEngine (diag-weight matmuls accumulating in PSUM,
shifts via the moving-operand access pattern), the VectorEngine (fused
scalar_tensor_tensor on fp32), and GPSIMD (scaled-copy + merge). Sigmoid on
ScalarE, which also evacuates PSUM into the z-buffer.
"""
import os
import numpy as np
import ml_dtypes

import concourse.bass as bass
import concourse.bacc as bacc
import concourse.tile as tile
import concourse.mybir as mybir
from concourse import bass_utils

N_CORES = 8
NLOC = 2            # samples per core
C = 256
H = W = 64
HW = H * W          # 4096
S = 3
PAD = 3
WP = W + 2 * PAD    # padded row width 70
HP = H + 2 * PAD
PSZ = WP * HP       # 70*70 = 4900 padded image size
CB = C // 128       # channel blocks per sample (2)
NU = NLOC * CB      # units per core (4)
PB = HW // 128      # pixel blocks per sample (32)
FQ = S * S          # 9
FKQ = C + FQ        # 265 fused feature columns
BN_EPS = 1e-5
BN_CNT = 16 * FQ    # 144 elements per channel in BN stats

# conv row-chunking: chunks of image rows; chunk spans <= 1536 px (3 banks)
CHUNKS = [(0, 24), (24, 24), (48, 16)]


def col_splits(span):
    out, off = [], 0
    while off < span:
        w = min(512, span - off)
        out.append((off, w))
        off += w
    return out


# per-dilation engine split of the 9 taps: PE via bf16 diag-matmul products,
# DVE via fused fp32 scalar_tensor_tensor, GPSIMD one scaled-copy tap
TAPS = {
    1: {"pe": (0, 1, 2, 3), "dve": (4, 5, 6, 7), "gp": (8,)},
    2: {"pe": (0, 1, 2, 3), "dve": (4, 5, 6, 7), "gp": (8,)},
    3: {"pe": (0, 1, 2), "dve": (3, 4, 5, 6, 7), "gp": (8,)},
}
# last unit: keep GPSIMD (slow TT merges) off the drain-out critical path
TAPS_LAST = TAPS
PE_TAP_SET = (0, 1, 2, 3)

dt = mybir.dt.float32
db = mybir.dt.bfloat16
ALU = mybir.AluOpType
AF = mybir.ActivationFunctionType
AX = mybir.AxisListType
BF = ml_dtypes.bfloat16


def tap_dydx(t, d):
    return d * (t // S - 1), d * (t % S - 1)


def _body(nc, tc, tens):
    x_d, xh_d, xl_d, wh_d, wl_d, bk_d, bq_d, g_d, b_d, out_d = tens
    with tc.tile_pool(name="const", bufs=1) as cpool, \
         tc.tile_pool(name="pimg", bufs=2) as ppool, \
         tc.tile_pool(name="pimgb", bufs=4) as pbpool, \
         tc.tile_pool(name="xcb", bufs=4) as xcpool, \
         tc.tile_pool(name="work", bufs=4) as wpool, \
         tc.tile_pool(name="big", bufs=2) as bpool, \
         tc.tile_pool(name="sig", bufs=3) as spool, \
         tc.tile_pool(name="diag", bufs=8) as gpool, \
         tc.tile_pool(name="small", bufs=1) as vpool, \
         tc.tile_pool(name="ps", bufs=2, space="PSUM") as ps, \
         tc.tile_pool(name="psk", bufs=1, space="PSUM") as psk, \
         tc.tile_pool(name="dram", bufs=2, space="DRAM") as dpool:

        # ---- constants / weights ----------------------------------------
        ident_d = nc.inline_tensor(np.eye(128, dtype=np.float32), name="ident")
        ident = cpool.tile([128, 128], dt, tag="ident")
        nc.sync.dma_start(ident[:], ident_d.ap())
        identb_d = nc.inline_tensor(np.eye(128, dtype=BF), name="identb")
        identb = cpool.tile([128, 128], db, tag="identb")
        nc.sync.dma_start(identb[:], identb_d.ap())

        wkq = {}
        for hl, w_d in (("h", wh_d), ("l", wl_d)):
            for ki in range(CB):
                t = cpool.tile([128, FKQ], db, tag=f"wkq{hl}{ki}",
                               name=f"wkq{hl}{ki}")
                nc.sync.dma_start(t[:], w_d.ap()[ki * 128:(ki + 1) * 128, :])
                wkq[(hl, ki)] = t

        ones_r = cpool.tile([1, 128], dt, tag="ones")
        nc.vector.memset(ones_r[:], 1.0)
        bias_r = cpool.tile([1, FKQ], dt, tag="biasr")
        nc.sync.dma_start(bias_r[0:1, 0:C],
                          bk_d.ap().rearrange("(p f) -> p f", p=1))
        nc.sync.dma_start(bias_r[0:1, C:C + FQ],
                          bq_d.ap().rearrange("(p f) -> p f", p=1))

        gam, bet = [], []
        for cb in range(CB):
            gt = cpool.tile([128, 1], dt, tag=f"g{cb}", name=f"g{cb}")
            bt = cpool.tile([128, 1], dt, tag=f"b{cb}", name=f"b{cb}")
            nc.sync.dma_start(
                gt[:], g_d.ap().rearrange("(p f) -> p f", f=1)[cb * 128:(cb + 1) * 128, :])
            nc.sync.dma_start(
                bt[:], b_d.ap().rearrange("(p f) -> p f", f=1)[cb * 128:(cb + 1) * 128, :])
            gam.append(gt)
            bet.append(bt)

        def pwin(pt, r0, nr, dy, dx):
            g = pt[:].rearrange("p (r c) -> p r c", c=WP)
            return g[:, PAD + r0 + dy:PAD + r0 + dy + nr,
                     PAD + dx:PAD + dx + W]

        # ---- features + per-sample kernel bmm ---------------------------
        # fkT[p,c'] = sum_c x[c,p] wkq^T[c,c'] + bias  (pixel-major)
        # krnl[c,t] = sum_p fkT[p,c] * fqT[p,t]
        bb_ps = ps.tile([128, FKQ], dt, tag="ps", name="bbps")
        nc.tensor.matmul(bb_ps[:], ones_r[:], bias_r[:], start=True, stop=True)
        bias_bc = cpool.tile([128, FKQ], dt, tag="biasbc")
        nc.vector.tensor_copy(bias_bc[:], bb_ps[:])

        krnl = [[vpool.tile([128, FQ], dt, tag=f"krnl{s}{cb}",
                            name=f"krnl{s}{cb}")
                 for cb in range(CB)] for s in range(NLOC)]
        for s in range(NLOC):
            xc = {}
            for cb in range(CB):
                for hl, src in (("h", xh_d), ("l", xl_d)):
                    t = xcpool.tile([128, HW], db, tag="xcb",
                                    name=f"xc{hl}{s}{cb}")
                    for q in range(2):
                        nc.sync.dma_start(
                            t[:, q * (HW // 2):(q + 1) * (HW // 2)],
                            src.ap()[s, cb * 128:(cb + 1) * 128].rearrange(
                                "p r c -> p (r c)")[:, q * (HW // 2):(q + 1) * (HW // 2)])
                    xc[(hl, cb)] = t
            kps = [psk.tile([128, FQ], dt, tag=f"kp{cb}", name=f"kp{s}{cb}")
                   for cb in range(CB)]
            for pb in range(PB):
                fp = ps.tile([128, FKQ], dt, tag="ps", name="feat")
                pr = [(xc[("h", ki)], wkq[("h", ki)]) for ki in range(CB)]
                pr += [(xc[("h", ki)], wkq[("l", ki)]) for ki in range(CB)]
                pr += [(xc[("l", ki)], wkq[("h", ki)]) for ki in range(CB)]
                for i, (xt, wt) in enumerate(pr):
                    nc.tensor.matmul(fp[:], xt[:, pb * 128:(pb + 1) * 128],
                                     wt[:], start=(i == 0),
                                     stop=(i == len(pr) - 1))
                fb = wpool.tile([128, FKQ], dt, tag="fkT", name="fb")
                nc.vector.tensor_tensor(out=fb[:], in0=fp[:], in1=bias_bc[:],
                                        op=ALU.add)
                for cb in range(CB):
                    nc.tensor.matmul(kps[cb][:],
                                     fb[:, cb * 128:(cb + 1) * 128],
                                     fb[:, C:C + FQ],
                                     start=(pb == 0), stop=(pb == PB - 1))
            for cb in range(CB):
                nc.vector.tensor_copy(krnl[s][cb][:], kps[cb][:])

        # ---- BN stats + AllReduce ---------------------------------------
        loc = []
        for cb in range(CB):
            st = vpool.tile([128, 2], dt, tag=f"st{cb}", name=f"st{cb}")
            tmp = wpool.tile([128, FQ], dt, tag="sq", name="sq")
            prt = wpool.tile([128, 4], dt, tag="prt", name="prt")
            for s in range(NLOC):
                nc.vector.tensor_reduce(prt[:, s:s + 1], krnl[s][cb][:],
                                        AX.X, ALU.add)
                nc.vector.tensor_tensor(out=tmp[:], in0=krnl[s][cb][:],
                                        in1=krnl[s][cb][:], op=ALU.mult)
                nc.vector.tensor_reduce(prt[:, 2 + s:3 + s], tmp[:],
                                        AX.X, ALU.add)
            nc.vector.tensor_tensor(out=st[:, 0:1], in0=prt[:, 0:1],
                                    in1=prt[:, 1:2], op=ALU.add)
            nc.vector.tensor_tensor(out=st[:, 1:2], in0=prt[:, 2:3],
                                    in1=prt[:, 3:4], op=ALU.add)
            loc.append(st)

        ib = dpool.tile([CB, 128, 2], dt)
        ob = dpool.tile([CB, 128, 2], dt)
        for cb in range(CB):
            nc.gpsimd.dma_start(ib[cb], loc[cb][:])
        if os.environ.get("PROF_NO_CC"):
            nc.gpsimd.dma_start(ob[:], ib[:])
        else:
            nc.gpsimd.collective_compute(
                "AllReduce", ALU.add, replica_groups=[list(range(N_CORES))],
                ins=[ib.opt()], outs=[ob.opt()])

        eps_t = vpool.tile([128, 1], dt, tag="eps")
        nc.vector.memset(eps_t[:], BN_EPS)
        scale, shift = [], []
        for cb in range(CB):
            gl = vpool.tile([128, 2], dt, tag=f"gl{cb}", name=f"gl{cb}")
            nc.gpsimd.dma_start(gl[:], ob[cb])
            mean = wpool.tile([128, 1], dt, tag="mean", name="mean")
            sc = vpool.tile([128, 1], dt, tag=f"sc{cb}", name=f"sc{cb}")
            sh = vpool.tile([128, 1], dt, tag=f"sh{cb}", name=f"sh{cb}")
            t0 = wpool.tile([128, 1], dt, tag="bn0", name="bn0")
            t1 = wpool.tile([128, 1], dt, tag="bn1", name="bn1")
            nc.vector.tensor_scalar_mul(mean[:], gl[:, 0:1], 1.0 / BN_CNT)
            nc.vector.tensor_tensor(out=t0[:], in0=mean[:], in1=mean[:],
                                    op=ALU.mult)
            nc.vector.scalar_tensor_tensor(
                out=t1[:], in0=gl[:, 1:2], scalar=1.0 / BN_CNT, in1=t0[:],
                op0=ALU.mult, op1=ALU.subtract)
            nc.scalar.activation(t0[:], t1[:], AF.Sqrt, bias=eps_t[:])
            nc.vector.reciprocal(t1[:], t0[:])
            nc.vector.tensor_tensor(out=sc[:], in0=gam[cb][:], in1=t1[:],
                                    op=ALU.mult)
            nc.vector.tensor_tensor(out=t0[:], in0=mean[:], in1=sc[:],
                                    op=ALU.mult)
            nc.vector.tensor_tensor(out=sh[:], in0=bet[cb][:], in1=t0[:],
                                    op=ALU.subtract)
            scale.append(sc)
            shift.append(sh)

        # normalized per-tap weights: w = krnl*scale + shift, plus a bf16
        # hi/lo split of w for the PE diag taps
        wnorm, wnlo = [], []
        for s in range(NLOC):
            wr, wr2 = [], []
            for cb in range(CB):
                wn = vpool.tile([128, FQ], dt, tag=f"wn{s}{cb}",
                                name=f"wn{s}{cb}")
                nc.vector.tensor_scalar(
                    out=wn[:], in0=krnl[s][cb][:],
                    scalar1=scale[cb][:], scalar2=shift[cb][:],
                    op0=ALU.mult, op1=ALU.add)
                wb = vpool.tile([128, FQ], db, tag=f"wb{s}{cb}",
                                name=f"wb{s}{cb}")
                nc.vector.tensor_copy(wb[:], wn[:])  # round to bf16
                wlo = vpool.tile([128, FQ], dt, tag=f"wlo{s}{cb}",
                                 name=f"wlo{s}{cb}")
                nc.vector.tensor_tensor(out=wlo[:], in0=wn[:], in1=wb[:],
                                        op=ALU.subtract)
                wlob = vpool.tile([128, FQ], db, tag=f"wlob{s}{cb}",
                                  name=f"wlob{s}{cb}")
                nc.vector.tensor_copy(wlob[:], wlo[:])
                wr.append((wn, wb, wlob))
                wr2.append(wlo)
            wnorm.append(wr)
            wnlo.append(wr2)

        # ---- depthwise convs + sigmoid + average ------------------------
        for u in range(NU):
            s, cb = divmod(u, CB)
            wn, wb, wlob = wnorm[s][cb]
            wlo = wnlo[s][cb]
            # reload this unit's x (fp32 + bf16 hi/lo) as zero-padded images
            pads = {}
            for key, src, dtt, pool, tg in (
                    ("f", x_d, dt, ppool, "pimg"),
                    ("h", xh_d, db, pbpool, "pimgb"),
                    ("l", xl_d, db, pbpool, "pimgb")):
                t = pool.tile([128, PSZ], dtt, tag=tg, name=f"pc{key}{u}")
                pg = t[:].rearrange("p (r c) -> p r c", c=WP)
                nc.vector.memset(t[:, 0:PAD * WP + PAD], 0.0)
                nc.vector.memset(t[:, PSZ - PAD * WP - PAD:PSZ], 0.0)
                nc.vector.memset(pg[:, PAD:PAD + H, 0:PAD], 0.0)
                nc.vector.memset(pg[:, PAD:PAD + H, PAD + W:WP], 0.0)
                nsp = 4 if key == "f" else 2
                rr = H // nsp
                for q in range(nsp):
                    nc.sync.dma_start(
                        pg[:, PAD + q * rr:PAD + (q + 1) * rr, PAD:PAD + W],
                        src.ap()[s, cb * 128:(cb + 1) * 128,
                                 q * rr:(q + 1) * rr])
                pads[key] = t
            diag = {}
            dgf_f32 = {}
            for t in PE_TAP_SET:
                dgf = gpool.tile([128, 128], dt, tag="diagf", name=f"dgf{u}_{t}")
                dgf_f32[t] = dgf
                nc.gpsimd.tensor_scalar_mul(dgf[:], ident[:], wn[:, t:t + 1])
                dgh = gpool.tile([128, 128], db, tag="diag",
                                 name=f"dgh{u}_{t}")
                nc.vector.tensor_copy(dgh[:], dgf[:])
                dgf2 = gpool.tile([128, 128], dt, tag="diagf",
                                  name=f"dgf2{u}_{t}")
                nc.gpsimd.tensor_scalar_mul(dgf2[:], ident[:],
                                            wlo[:, t:t + 1])
                dgl = gpool.tile([128, 128], db, tag="diag",
                                 name=f"dgl{u}_{t}")
                nc.vector.tensor_copy(dgl[:], dgf2[:])
                diag[t] = (dgh, dgl)
            last = (u == NU - 1)
            sigacc = None
            for di, d in enumerate((1, 2, 3)):
                cfg = TAPS_LAST[d] if last else TAPS[d]
                zb = bpool.tile([128, HW], dt, tag="big", name=f"zb{u}_{di}")
                zb3 = zb[:].rearrange("p (r c) -> p r c", c=W)
                for (r0, nr) in CHUNKS:
                    span = nr * W
                    pz = ps.tile([128, span], dt, tag="ps", name=f"pz{di}")
                    prods = []
                    for t in cfg["pe"]:
                        dy, dx = tap_dydx(t, d)
                        dgh, dgl = diag[t]
                        prods += [(dgh, "h", dy, dx), (dgh, "l", dy, dx),
                                  (dgl, "h", dy, dx)]
                    for i, (dg, key, dy, dx) in enumerate(prods):
                        for (off, wdt) in col_splits(span):
                            nc.tensor.matmul(
                                pz[:, off:off + wdt], dg[:],
                                pwin(pads[key], r0 + off // W, wdt // W,
                                     dy, dx),
                                start=(i == 0), stop=(i == len(prods) - 1))
                    nc.scalar.copy(zb[:, r0 * W:(r0 + nr) * W], pz[:])
                for t in cfg["dve"]:
                    dy, dx = tap_dydx(t, d)
                    nc.vector.scalar_tensor_tensor(
                        out=zb3, in0=pwin(pads["f"], 0, H, dy, dx),
                        scalar=wn[:, t:t + 1], in1=zb3,
                        op0=ALU.mult, op1=ALU.add)
                if cfg["gp"]:
                    ga = spool.tile([128, HW], dt, tag="sig",
                                    name=f"ga{u}_{di}")
                    for t in cfg["gp"]:
                        dy, dx = tap_dydx(t, d)
                        nc.gpsimd.tensor_scalar_mul(
                            ga[:].rearrange("p (r c) -> p r c", c=W),
                            pwin(pads["f"], 0, H, dy, dx), wn[:, t:t + 1])
                    nc.gpsimd.tensor_tensor(out=zb[:], in0=zb[:], in1=ga[:],
                                            op=ALU.add)
                sg = spool.tile([128, HW], dt, tag="sig", name=f"sg{u}_{di}")
                nc.scalar.activation(sg[:], zb[:], AF.Sigmoid)
                if di == 0:
                    sigacc = sg
                else:
                    nc.gpsimd.tensor_tensor(out=sigacc[:], in0=sigacc[:],
                                            in1=sg[:], op=ALU.add)
            ext = spool.tile([128, HW], dt, tag="sig", name=f"ext{u}")
            nc.scalar.activation(ext[:], sigacc[:], AF.Copy, scale=1.0 / 3.0)
            for q in range(2):
                nc.sync.dma_start(
                    out_d.ap()[s, cb * 128:(cb + 1) * 128,
                               q * 32:(q + 1) * 32],
                    ext[:].rearrange("p (r c) -> p r c", c=W)[:, q * 32:(q + 1) * 32, :])


def _build():
    nc = bacc.Bacc("TRN2", debug=False, num_devices=N_CORES,
                   target_bir_lowering=False)
    x_d = nc.dram_tensor("x", [NLOC, C, H, W], dt, kind="ExternalInput")
    xh_d = nc.dram_tensor("xh", [NLOC, C, H, W], db, kind="ExternalInput")
    xl_d = nc.dram_tensor("xl", [NLOC, C, H, W], db, kind="ExternalInput")
    wh_d = nc.dram_tensor("wkqth", [C, FKQ], db, kind="ExternalInput")
    wl_d = nc.dram_tensor("wkqtl", [C, FKQ], db, kind="ExternalInput")
    bk_d = nc.dram_tensor("bk", [C], dt, kind="ExternalInput")
    bq_d = nc.dram_tensor("bq", [FQ], dt, kind="ExternalInput")
    g_d = nc.dram_tensor("gamma", [C], dt, kind="ExternalInput")
    b_d = nc.dram_tensor("beta", [C], dt, kind="ExternalInput")
    out_d = nc.dram_tensor("out", [NLOC, C, H, W], dt, kind="ExternalOutput")
    with tile.TileContext(nc) as tc:
        _body(nc, tc, (x_d, xh_d, xl_d, wh_d, wl_d, bk_d, bq_d, g_d, b_d,
                       out_d))
    nc.compile()
    return nc


_nc_cache = None
last_results = None


def kernel(x, wk, bk, wq, bq, gamma, beta):
    global _nc_cache, last_results
    if _nc_cache is None:
        _nc_cache = _build()
    nc = _nc_cache
    x = np.ascontiguousarray(x, dtype=np.float32)
    xh = x.astype(BF)
    xl = (x - xh.astype(np.float32)).astype(BF)
    wkqt = np.concatenate(
        [np.asarray(wk, np.float32).T, np.asarray(wq, np.float32).T],
        axis=1)  # [C, 265]
    wh = wkqt.astype(BF)
    wl = (wkqt - wh.astype(np.float32)).astype(BF)
    in_maps = []
    for c in range(N_CORES):
        sl = slice(c * NLOC, (c + 1) * NLOC)
        in_maps.append({
            "x": x[sl], "xh": np.ascontiguousarray(xh[sl]),
            "xl": np.ascontiguousarray(xl[sl]),
            "wkqth": np.ascontiguousarray(wh),
            "wkqtl": np.ascontiguousarray(wl),
            "bk": np.ascontiguousarray(bk, np.float32),
            "bq": np.ascontiguousarray(bq, np.float32),
            "gamma": np.ascontiguousarray(gamma, np.float32),
            "beta": np.ascontiguousarray(beta, np.float32),
        })
    res = bass_utils.run_bass_kernel_spmd(
        nc, in_maps, core_ids=list(range(N_CORES)))
    last_results = res
    out = np.concatenate([res.results[c]["out"] for c in range(N_CORES)],
                         axis=0)
    return out
